# revision 20
# baseline (speedup 1.0000x reference)
"""Trainium2 Bass kernel for nn_DependencyEncoder (stack TreeLSTM).

Self-contained: takes FULL inputs as in reference.setup_inputs(), shards the
batch across 8 NeuronCores (pure data parallelism), runs a fully static
Bass/Tile program specialized on the (batch-uniform) transition schedule,
and gathers the full [B, H] output.

Device program layout (per core, b = B/8 examples):
- Everything feature-on-partition, batch on the free dim.
- tokens_h^T / tokens_c^T as [128, 2*L*b] bf16, free = h1*(L*b) + t*b + e.
- Track gates row order (i, f, 2g, o), PSUM *window* tiles [TD, 4*2*b]
  covering two consecutive steps; token-side fills are run-grouped into
  N=128 matmuls (ascending/descending/broadcast AP views of the token
  buffer).  tanh(g) = 2*sigmoid(2g)-1 with the 2x pre-folded into weights.
- Tree gates row order (2u, i, f_l, f_r, o), PSUM *pair* tiles
  [128, 10*2*b] covering reduce steps (r, r+2) which share the same U
  matrix (U_l for even r, U_r for odd); token-side fills become N=128
  matmuls over stride -2 token views.
- Biases ride augmented matmuls: th state tile is [TD+1, b] with last row 1,
  multiplied by [W_hh^T; b_ih+b_hh] and [W_x^T; b_l].
- PE emission order keeps the in-order PE busy through the serial
  ACT/DVE chain windows (fills + small dummies) so HAM holds K=8/8.
"""

import os
import sys

os.environ.setdefault("JAX_PLATFORMS", "")
if "/opt/trn_rl_repo" not in sys.path:
    sys.path.insert(0, "/opt/trn_rl_repo")

import numpy as np
import ml_dtypes

BF16 = ml_dtypes.bfloat16
N_CORES = 8
H = 256
TD = 64

# ---------------------------------------------------------------- schedule --

# Track gate rows: original (i, f, g, o); device order (i, f, 2g, o).
_TRACK_PERM = np.arange(256)
_TRACK_SCALE = np.concatenate(
    [np.ones(128), np.full(64, 2.0), np.ones(64)]).astype(np.float32)
# Tree gate rows: original (i, o, f_l, f_r, u); device (2u, i, f_l, f_r, o).
_TREE_PERM = np.concatenate([np.arange(4 * H, 5 * H), np.arange(0, H),
                             np.arange(2 * H, 3 * H), np.arange(3 * H, 4 * H),
                             np.arange(H, 2 * H)])
_TREE_SCALE = np.concatenate([np.full(H, 2.0), np.ones(4 * H)]).astype(np.float32)


def derive_schedule(transitions: np.ndarray, L: int):
    """Symbolic stack simulation over the batch-uniform transition codes."""
    tr = np.asarray(transitions)
    if not (tr == tr[0:1]).all():
        raise NotImplementedError("non-batch-uniform transitions unsupported")
    codes = [int(c) for c in tr[0]]
    MAX_STACK = L + 2
    stack = [("tok", 0), ("tok", 0)] + [None] * (MAX_STACK - 2)
    p, bp, nred = 2, 0, 0
    steps = []
    for c in codes:
        assert 2 <= p <= MAX_STACK, f"invalid stack pointer {p}"
        top = stack[p - 1]
        sec = stack[p - 2]
        buf = ("tok", min(bp, L - 1))
        is_shift = c == 1
        is_red = c in (2, 3)
        step = dict(code=c, buf=buf, top=top, sec=sec, is_red=is_red,
                    head=None, chil=None, red_idx=None)
        if is_red:
            head, chil = (top, sec) if c == 2 else (sec, top)
            val = ("red", nred)
            step.update(head=head, chil=chil, red_idx=nred)
            nred += 1
        elif is_shift:
            val = buf
        else:
            val = top
        pos = p if is_shift else (p - 2 if is_red else p - 1)
        assert 0 <= pos < MAX_STACK
        stack[pos] = val
        p = p + int(is_shift) - int(is_red)
        bp = bp + int(is_shift)
        steps.append(step)
    return steps, stack[p - 1]


# ------------------------------------------------------------ host packing --

def _chunk_k(wt: np.ndarray) -> np.ndarray:
    """[K, M] -> [128, (K//128)*M], K-chunks stacked along the free dim."""
    K = wt.shape[0]
    assert K % 128 == 0
    return np.hstack([wt[k * 128:(k + 1) * 128] for k in range(K // 128)])


def prep_weights(W_x, U_r, U_l, b_l, W_ih, W_hh, b_ih, b_hh):
    W_ih = np.asarray(W_ih, np.float32)
    W_hh = np.asarray(W_hh, np.float32)
    sc = _TRACK_SCALE[:, None]
    W_A = W_ih[:, 0:H][_TRACK_PERM] * sc
    W_B = W_ih[:, H:2 * H][_TRACK_PERM] * sc
    W_C = W_ih[:, 2 * H:3 * H][_TRACK_PERM] * sc
    W_hh_p = W_hh[_TRACK_PERM] * sc
    btot = ((np.asarray(b_ih) + np.asarray(b_hh))[_TRACK_PERM] * _TRACK_SCALE)

    tsc = _TREE_SCALE[:, None]
    U_l_p = np.asarray(U_l, np.float32)[_TREE_PERM] * tsc
    U_r_p = np.asarray(U_r, np.float32)[_TREE_PERM] * tsc
    W_x_p = np.asarray(W_x, np.float32)[_TREE_PERM] * tsc
    b_l_p = np.asarray(b_l, np.float32)[_TREE_PERM] * _TREE_SCALE
    out = dict(
        wa=_chunk_k(np.ascontiguousarray(W_A.T)),                    # [128, 512]
        wb=_chunk_k(np.ascontiguousarray(W_B.T)),
        wc=_chunk_k(np.ascontiguousarray(W_C.T)),
        whh=np.vstack([W_hh_p.T, btot[None, :]]),                    # [65, 256]
        ul=_chunk_k(np.ascontiguousarray(U_l_p.T)),
        ur=_chunk_k(np.ascontiguousarray(U_r_p.T)),
        wx=np.vstack([W_x_p.T, b_l_p[None, :]]),                     # [65, 1280]
        ident=np.eye(128, dtype=np.float32),
    )
    return {k: np.ascontiguousarray(v, dtype=BF16) for k, v in out.items()}


def prep_tokens(tokens: np.ndarray, dtype=BF16) -> np.ndarray:
    """[b, L, H] -> [128, 2*L*b], free = h1*(L*b) + t*b + e."""
    b, L, Hn = tokens.shape
    assert Hn == H
    arr = np.asarray(tokens, np.float32).transpose(2, 1, 0).reshape(H, L * b)
    return np.ascontiguousarray(np.hstack([arr[:128], arr[128:]]).astype(dtype))


def _group_runs(syms):
    """[(kind, idx)] -> [(s0, ln, idx0, delta)] over tok syms; delta in
    {-2,-1,0,1,2} (run of 1 gets delta 0)."""
    runs = []
    i = 0
    n = len(syms)
    while i < n:
        if syms[i][0] != "tok":
            i += 1
            continue
        j = i + 1
        delta = None
        while j < n and syms[j][0] == "tok":
            d = syms[j][1] - syms[j - 1][1]
            if delta is None:
                if d in (-2, -1, 0, 1, 2):
                    delta = d
                else:
                    break
            elif d != delta:
                break
            j += 1
        runs.append((i, j - i, syms[i][1], delta if delta is not None else 0))
        i = j
    return runs


# ---------------------------------------------------------- device program --

WTRK = 2  # track PSUM window size (steps per tile)


def _build_program(steps, out_sym, b, L, n_dummy=0):
    import concourse.bacc as bacc
    import concourse.mybir as mybir
    import concourse.tile as tile

    f32 = mybir.dt.float32
    bf16 = mybir.dt.bfloat16
    AF = mybir.ActivationFunctionType
    nc = bacc.Bacc("TRN2", target_bir_lowering=False, debug=False)
    Lb = L * b
    nT = len(steps)

    tc_dt = f32 if int(os.environ.get("KERNEL_TCF32", "1")) else bf16
    d = {}
    DTYPES = {"tc0": tc_dt}
    for name, shape in [
        ("tokh", [128, 2 * Lb]), ("tokc", [128, 2 * Lb]),
        ("wa", [128, 512]), ("wb", [128, 512]), ("wc", [128, 512]),
        ("whh", [TD + 1, 256]),
        ("ul", [128, 2 * 1280]), ("ur", [128, 2 * 1280]),
        ("wx", [TD + 1, 1280]),
        ("th0", [TD, b]), ("tc0", [TD, b]), ("ident", [128, 128]),
    ]:
        d[name] = nc.declare_dram_parameter(name, shape, DTYPES.get(name, bf16),
                                            isOutput=False)
    d_out = nc.declare_dram_parameter("out", [b, H], f32, isOutput=True)

    # ---- tree pair structure: reduce index r -> (pair base, slot) --------
    red_steps = [t for t in range(nT) if steps[t]["is_red"]]
    rix = {t: i for i, t in enumerate(red_steps)}  # step -> reduce index

    def pair_base(r):
        return 4 * (r // 4) + (r % 2)

    def pair_slot(r):
        return (r % 4) // 2

    def tok_role(r):
        # operand that is (normally) a token for this reduce index
        return "chil" if r % 2 == 0 else "head"

    def red_role(r):
        return "head" if r % 2 == 0 else "chil"

    with tile.TileContext(nc) as tc:
        with (
            tc.tile_pool(name="const", bufs=1) as cp,
            tc.tile_pool(name="wk", bufs=3) as wp,
            tc.tile_pool(name="pstr", bufs=2, space="PSUM") as pstr,
            tc.tile_pool(name="psgt", bufs=2, space="PSUM") as psgt,
        ):
            sb = {}
            for name in ("tokh", "tokc", "wa", "wb", "wc", "whh", "ul", "ur",
                         "wx", "ident"):
                sb[name] = cp.tile(list(d[name].shape), DTYPES.get(name, bf16),
                                   name=f"sb_{name}")
                nc.sync.dma_start(sb[name][:], d[name].ap())

            # ping-pong state tiles; th has an extra all-ones row (bias mule)
            th_t = [cp.tile([TD + 1, b], bf16, name=f"th{i}") for i in range(2)]
            tc_t = [cp.tile([TD, b], tc_dt, name=f"tcs{i}") for i in range(2)]
            for i in range(2):
                nc.vector.memset(th_t[i][TD:TD + 1, :], 1.0)
            nc.sync.dma_start(th_t[0][0:TD, :], d["th0"].ap())
            nc.sync.dma_start(tc_t[0][:], d["tc0"].ap())

            tokh, tokc = sb["tokh"], sb["tokc"]
            tokv = tokh[:].rearrange("p (k l b) -> p k l b", k=2, b=b)
            red_h, red_c = {}, {}

            def h_rhs(sym, k):
                kind, idx = sym
                if kind == "tok":
                    return tokh[:, k * Lb + idx * b: k * Lb + (idx + 1) * b]
                return red_h[idx][:, k * b:(k + 1) * b]

            def c_view(sym):
                kind, idx = sym
                if kind == "tok":
                    v = tokc[:].rearrange("p (k l b) -> p k l b", k=2, b=b)
                    return v[:, :, idx, :]
                return red_c[idx][:].rearrange("p (k b) -> p k b", k=2)

            def tok_run_view(k, idx0, ln, delta):
                if ln == 1:
                    return tokv[:, k, idx0, :]
                if delta == 1:
                    return tokv[:, k, idx0:idx0 + ln, :]
                if delta == 0:
                    return (tokv[:, k, idx0, :].unsqueeze(1)
                            .broadcast_to([128, ln, b]))
                lo = idx0 + delta * ln  # delta < 0
                return tokv[:, k, idx0:(lo if lo >= 0 else None):delta, :]

            # ---------------- PSUM tile state ---------------------------
            trk_state = {}   # window id -> (tile, started, t0, sz)

            def track_ps(t):
                w = t // WTRK
                if w not in trk_state:
                    t0 = w * WTRK
                    sz = min(WTRK, nT - t0)
                    tile_ = pstr.tile([TD, 4 * sz * b], f32,
                                      name=f"pstr_{w}", tag="pstr")
                    trk_state[w] = (tile_, set(), t0, sz)
                return trk_state[w]

            def trk_out(t, m, ln=1):
                tile_, started, t0, sz = track_ps(t)
                s = t - t0
                c0 = m * (sz * b) + s * b
                return tile_[:, c0:c0 + ln * b], started

            def trk_gate_view(t, g0, gn):
                tile_, _, t0, sz = track_ps(t)
                v = tile_[:].rearrange("p (m s b) -> p m s b", m=4, b=b)
                return v[:, g0:g0 + gn, t - t0, :]

            pair_state = {}  # pair base -> (tile, started_banks, rs)

            def tree_ps(r):
                pb = pair_base(r)
                if pb not in pair_state:
                    rs = [x for x in (pb, pb + 2) if x < len(red_steps)]
                    tile_ = psgt.tile([128, 10 * 2 * b], f32,
                                      name=f"psg_{pb}", tag="psg")
                    pair_state[pb] = (tile_, set(), rs)
                return pair_state[pb]

            def tree_out(r, m, ln=1):
                tile_, started, rs = tree_ps(r)
                s = pair_slot(r)
                c0 = m * (2 * b) + s * b
                return tile_[:, c0:c0 + ln * b], started, (m * 2 * b * 4) // 2048

            def tree_gate_view(r, m0, mn):
                tile_, _, _ = tree_ps(r)
                v = tile_[:].rearrange("p (m s b) -> p m s b", m=10, b=b)
                return v[:, m0:m0 + mn, pair_slot(r), :]

            # ---------------- emission helpers --------------------------
            def emit_track_window_fills(w):
                t0 = w * WTRK
                sz = min(WTRK, nT - t0)
                tile_, started, _, _ = track_ps(t0)
                for w_t, role in ((sb["wa"], "buf"), (sb["wc"], "sec"),
                                  (sb["wb"], "top")):
                    syms = [steps[t0 + s][role] for s in range(sz)]
                    for (s0, ln, idx0, delta) in _group_runs(syms):
                        for m in range(4):
                            for k in range(2):
                                out = tile_[:, m * (sz * b) + s0 * b:
                                            m * (sz * b) + (s0 + ln) * b]
                                nc.tensor.matmul(
                                    out,
                                    w_t[:, k * 256 + m * 64:
                                        k * 256 + (m + 1) * 64],
                                    tok_run_view(k, idx0, ln, delta),
                                    start=0 not in started, stop=False,
                                    skip_group_check=True)
                                started.add(0)

            def emit_track_red(t):
                # stack-top operand when it is a previous reduce result
                st = steps[t]
                for w_t, role in ((sb["wb"], "top"), (sb["wc"], "sec"),
                                  (sb["wa"], "buf")):
                    sym = st[role]
                    if sym[0] != "red":
                        continue
                    for m in range(4):
                        out, started = trk_out(t, m)
                        for k in range(2):
                            nc.tensor.matmul(
                                out,
                                w_t[:, k * 256 + m * 64: k * 256 + (m + 1) * 64],
                                h_rhs(sym, k), start=0 not in started,
                                stop=False, skip_group_check=True)
                            started.add(0)

            def emit_whh(t):
                for m in range(4):
                    out, started = trk_out(t, m)
                    nc.tensor.matmul(out, sb["whh"][:, m * 64:(m + 1) * 64],
                                     th_t[t % 2][:], start=0 not in started,
                                     stop=(m == 3), skip_group_check=True)
                    started.add(0)

            def emit_tree_pair_fills(pb):
                tile_, started, rs = tree_ps(pb)
                w_t = sb["ul"] if pb % 2 == 0 else sb["ur"]
                syms = [steps[red_steps[r]][tok_role(r)] for r in rs]
                for (s0, ln, idx0, delta) in _group_runs(syms):
                    for m in range(10):
                        bank = (m * 2 * b * 4) // 2048
                        for k in range(2):
                            out = tile_[:, m * (2 * b) + s0 * b:
                                        m * (2 * b) + (s0 + ln) * b]
                            nc.tensor.matmul(
                                out,
                                w_t[:, k * 1280 + m * 128:
                                    k * 1280 + (m + 1) * 128],
                                tok_run_view(k, idx0, ln, delta),
                                start=bank not in started, stop=False,
                                skip_group_check=True)
                            started.add(bank)

            def emit_tree_extra_tok(r):
                # token operand in the "red" role (e.g. r=0 head=tok62)
                st = steps[red_steps[r]]
                sym = st[red_role(r)]
                if sym[0] != "tok":
                    return
                w_t = sb["ur"] if r % 2 == 0 else sb["ul"]
                # note: red_role even r = head -> U_r; odd = chil -> U_l
                for m in range(10):
                    out, started, bank = tree_out(r, m)
                    for k in range(2):
                        nc.tensor.matmul(
                            out,
                            w_t[:, k * 1280 + m * 128: k * 1280 + (m + 1) * 128],
                            h_rhs(sym, k), start=bank not in started,
                            stop=False, skip_group_check=True)
                        started.add(bank)

            def emit_tree_red(r):
                st = steps[red_steps[r]]
                sym = st[red_role(r)]
                if sym[0] != "red":
                    return
                w_t = sb["ur"] if r % 2 == 0 else sb["ul"]
                for m in range(10):
                    out, started, bank = tree_out(r, m)
                    for k in range(2):
                        nc.tensor.matmul(
                            out,
                            w_t[:, k * 1280 + m * 128: k * 1280 + (m + 1) * 128],
                            h_rhs(sym, k), start=bank not in started,
                            stop=False, skip_group_check=True)
                        started.add(bank)

            def emit_wx(r):
                t = red_steps[r]
                tile_, started, rs = tree_ps(r)
                last = r == rs[-1]
                for m in range(10):
                    out, _, bank = tree_out(r, m)
                    nc.tensor.matmul(out, sb["wx"][:, m * 128:(m + 1) * 128],
                                     th_t[(t + 1) % 2][:],
                                     start=bank not in started,
                                     stop=(last and m == 9),
                                     skip_group_check=True)
                    started.add(bank)

            # fp32 const tile for LDWEIGHTS-only fillers (no PSUM, no sems:
            # pure PE-array busy work so HAM holds K=8/8 through the serial
            # ACT/DVE chain windows)
            dmw = cp.tile([128, 64], bf16, name="dmw")
            nc.vector.memset(dmw[:], 1.0)

            def emit_dummies(t, n):
                for i in range(n):
                    nc.tensor.ldweights(dmw[:])

            def emit_warmup(n):
                # real matmuls (guaranteed HAM-visible) into a rotating
                # track-pool tile, emitted before the first window fills
                for i in range(n):
                    wt_ = pstr.tile([64, 128], f32, tag="pstr",
                                    name=f"warm_{i}")
                    nc.tensor.matmul(wt_[:], sb["wa"][:, 0:64],
                                     sb["wb"][:, 0:128], start=True,
                                     stop=True, skip_group_check=True)

            def emit_track_elem(t):
                cur, nxt = t % 2, (t + 1) % 2
                # sigmoid over (i,f,2g) unblocks the chain; sigma(o) follows
                sig = wp.tile([TD, 4 * b], bf16, tag="sig", name=f"sig_{t}")
                nc.scalar.activation(sig[:, 0:3 * b], trk_gate_view(t, 0, 3),
                                     AF.Sigmoid)
                nc.scalar.activation(sig[:, 3 * b:4 * b],
                                     trk_gate_view(t, 3, 1), AF.Sigmoid)
                si, sf = sig[:, 0:b], sig[:, b:2 * b]
                s2g, so = sig[:, 2 * b:3 * b], sig[:, 3 * b:4 * b]
                At = wp.tile([TD, b], bf16, tag="At", name=f"At_{t}")
                jk = wp.tile([TD, 1], f32, tag="jk", name=f"jk_{t}")
                nc.vector.affine_mul_reduce(At[:], jk[:], s2g, si, 2.0, -1.0)
                Bt = wp.tile([TD, b], tc_dt, tag="Bt", name=f"Bt_{t}")
                nc.vector.tensor_mul(Bt[:], sf, tc_t[cur][:])
                nc.vector.tensor_add(tc_t[nxt][:], At[:], Bt[:])
                tt = wp.tile([TD, b], bf16, tag="tt", name=f"tt_{t}")
                nc.scalar.activation(tt[:], tc_t[nxt][:], AF.Tanh)
                nc.vector.tensor_mul(th_t[nxt][0:TD, :], tt[:], so)

            def emit_tree_elem(r):
                st = steps[red_steps[r]]
                sg = wp.tile([128, 10 * b], bf16, tag="sg", name=f"sg_{r}")
                nc.scalar.activation(sg[:, 0:8 * b], tree_gate_view(r, 0, 8),
                                     AF.Sigmoid)
                nc.scalar.activation(sg[:, 8 * b:10 * b],
                                     tree_gate_view(r, 8, 2), AF.Sigmoid)
                s2u, sgi = sg[:, 0:2 * b], sg[:, 2 * b:4 * b]
                sfl, sfr = sg[:, 4 * b:6 * b], sg[:, 6 * b:8 * b]
                sgo = sg[:, 8 * b:10 * b]
                r3 = lambda ap: ap.rearrange("p (k b) -> p k b", k=2)
                A2 = wp.tile([128, 2 * b], bf16, tag="A2", name=f"A2_{r}")
                jk2 = wp.tile([128, 1], f32, tag="jk2", name=f"jk2_{r}")
                nc.vector.affine_mul_reduce(A2[:], jk2[:], s2u, sgi, 2.0, -1.0)
                B2 = wp.tile([128, 2 * b], bf16, tag="B2", name=f"B2_{r}")
                nc.vector.tensor_mul(r3(B2[:]), r3(sfl), c_view(st["chil"]))
                C2 = wp.tile([128, 2 * b], bf16, tag="C2", name=f"C2_{r}")
                nc.vector.tensor_mul(r3(C2[:]), r3(sfr), c_view(st["head"]))
                S2 = wp.tile([128, 2 * b], bf16, tag="S2", name=f"S2_{r}")
                nc.vector.tensor_add(S2[:], A2[:], B2[:])
                rc = wp.tile([128, 2 * b], bf16, tag="rc", name=f"rc_{r}")
                nc.vector.tensor_add(rc[:], S2[:], C2[:])
                tt2 = wp.tile([128, 2 * b], bf16, tag="tt2", name=f"tt2_{r}")
                nc.scalar.activation(tt2[:], rc[:], AF.Tanh)
                rh = wp.tile([128, 2 * b], bf16, tag="rh", name=f"rh_{r}")
                nc.vector.tensor_mul(rh[:], tt2[:], sgo)
                red_h[st["red_idx"]] = rh
                red_c[st["red_idx"]] = rc

            # ---------------- main emission loop -------------------------
            n_prewx = int(os.environ.get("KERNEL_PREWX", "0"))
            n_warm = int(os.environ.get("KERNEL_WARMUP", "0"))
            if n_warm:
                emit_warmup(n_warm)
            emit_track_window_fills(0)
            emit_whh(0)
            if steps[0]["is_red"]:
                emit_tree_pair_fills(pair_base(rix[0]))
                emit_tree_extra_tok(rix[0])
            for t in range(nT):
                st = steps[t]
                is_red = st["is_red"]
                r = rix.get(t)
                emit_track_red(t)
                if is_red:
                    emit_tree_red(r)
                emit_track_elem(t)
                if n_prewx and is_red:
                    emit_dummies(t, n_prewx)
                if is_red:
                    emit_wx(r)
                if t + 1 < nT:
                    if (t + 1) // WTRK != t // WTRK:
                        emit_track_window_fills((t + 1) // WTRK)
                    emit_whh(t + 1)
                    r1 = rix.get(t + 1)
                    if r1 is not None:
                        if pair_base(r1) not in pair_state:
                            emit_tree_pair_fills(pair_base(r1))
                        emit_tree_extra_tok(r1)
                emit_dummies(t, n_dummy if is_red else max(n_dummy - 3, 0))
                if is_red:
                    emit_tree_elem(r)

            # ---- output: transpose [H, b] -> [b, H] and store ----
            out_sb = wp.tile([b, H], f32, tag="out", name="out_sb")
            for k in range(2):
                pot = pstr.tile([b, 128], bf16, tag="pstr", name=f"pout_{k}")
                nc.tensor.transpose(pot[:], h_rhs(out_sym, k), sb["ident"][:])
                nc.scalar.copy(out_sb[:, k * 128:(k + 1) * 128], pot[:])
            nc.sync.dma_start(d_out.ap(), out_sb[:])

    nc.compile()
    return nc


_PROGRAM_CACHE = {}


def _get_program(codes_key, b, L, steps, out_sym):
    nd = int(os.environ.get("KERNEL_NDUMMY", "0"))
    knobs = tuple(os.environ.get(k, "") for k in
                  ("KERNEL_PREWX", "KERNEL_WARMUP", "KERNEL_TCF32"))
    key = (codes_key, b, L, nd, knobs)
    if key not in _PROGRAM_CACHE:
        _PROGRAM_CACHE[key] = _build_program(steps, out_sym, b, L, n_dummy=nd)
    return _PROGRAM_CACHE[key]


# ------------------------------------------------------------------ kernel --

def kernel(**inputs) -> np.ndarray:
    from concourse.bass_utils import run_bass_kernel_spmd

    tokens_h = np.asarray(inputs["tokens_h"], np.float32)
    tokens_c = np.asarray(inputs["tokens_c"], np.float32)
    transitions = np.asarray(inputs["transitions"])
    th0 = np.asarray(inputs["th0"], np.float32)
    tc0 = np.asarray(inputs["tc0"], np.float32)
    B, L, Hn = tokens_h.shape
    assert Hn == H and B % N_CORES == 0
    b = B // N_CORES

    steps, out_sym = derive_schedule(transitions, L)
    codes_key = tuple(int(c) for c in transitions[0])
    nc = _get_program(codes_key, b, L, steps, out_sym)

    w = prep_weights(inputs["W_x"], inputs["U_r"], inputs["U_l"], inputs["b_l"],
                     inputs["W_ih"], inputs["W_hh"], inputs["b_ih"], inputs["b_hh"])
    in_maps = []
    for core in range(N_CORES):
        sl = slice(core * b, (core + 1) * b)
        m = dict(w)
        m["tokh"] = prep_tokens(tokens_h[sl], BF16)
        m["tokc"] = prep_tokens(tokens_c[sl], BF16)
        m["th0"] = np.ascontiguousarray(th0[sl].T.astype(BF16))
        tc_np = (np.float32 if int(os.environ.get("KERNEL_TCF32", "1"))
                 else BF16)
        m["tc0"] = np.ascontiguousarray(tc0[sl].T.astype(tc_np))
        in_maps.append(m)

    trace = bool(int(os.environ.get("KERNEL_TRACE", "0")))
    res = run_bass_kernel_spmd(nc, in_maps, list(range(N_CORES)), trace=trace)
    if trace:
        kernel.last_exec_time_ns = res.exec_time_ns
        kernel.last_results = res
    out = np.concatenate([res.results[i]["out"] for i in range(N_CORES)], axis=0)
    return np.ascontiguousarray(out, dtype=np.float32)
